# revision 1
# baseline (speedup 1.0000x reference)
"""Causal multi-head self-attention (B=4, S=2048, D=1024, 16 heads) on 8 TRN2 cores.

Sharding: core c -> batch b = c//2, head-half hh = c%2 (8 of 16 heads, 512 of
1024 projection dims).  Each core:
  - projects its batch's q/k/v against its 512 weight columns (the 1/sqrt(D)
    score scale is folded into Wq/bq on the host),
  - runs causal attention for its 8 heads in a scores-transposed layout
    (keys on partitions, queries on the free dim): head pairs share one kT/qT
    tile at partition offsets 0/64 so their K=64 scores matmuls run in
    different PE row groups concurrently; exp on ScalarE (one strided
    activation per head pair); softmax denominators via an appended
    ones-column in V; normalization by a K=1 outer-product broadcast of
    1/sums multiplied in on VectorE (writes ctxT directly, no transposes),
  - applies its 512 rows of Wo (bias bo/2 added via a K=1 ones matmul),
  - per-512-row-slab pairwise ReduceScatter sums the two half-head partials
    (overlapping the collective with later chunks); each core ends with half
    the sequence rows of its batch's output, interleaved by slab.
Matmuls run as fp32r (full-rate PE); q/k/v/p are bf16 (scores + ctx matmuls
bf16) -- measured ~2e-3 relative error.
"""

import numpy as np

B = 4
S = 2048
DM = 1024
HD = 64
NH = 8            # heads per core
OD = NH * HD      # 512: per-core projection width
NCORES = 8
QCN = S // 512    # 4 query chunks of 512
DBLK = DM // 128  # 8 contraction blocks
OBLK = OD // 128  # 4 output-dim blocks
SBLK = S // 128   # 16 seq strips

_prog = None


def _build_program(repeat=1):
    from contextlib import ExitStack
    from concourse import bacc, mybir
    import concourse.tile as tile

    f32 = mybir.dt.float32
    f32r = mybir.dt.float32r
    bf16 = mybir.dt.bfloat16
    EXP = mybir.ActivationFunctionType.Exp
    IDENT = mybir.ActivationFunctionType.Identity
    COPY = mybir.ActivationFunctionType.Copy

    nc = bacc.Bacc(None, num_devices=NCORES)

    # --- external I/O (per-core shards) ---
    xq_ext = nc.declare_dram_parameter("xq", [DM, S], f32r, isOutput=False)
    xk_ext = nc.declare_dram_parameter("xk", [DM, S], f32r, isOutput=False)
    xv_ext = nc.declare_dram_parameter("xv", [DM, S], f32r, isOutput=False)
    wq_ext = nc.declare_dram_parameter("wq", [DM, OD], f32r, isOutput=False)
    wk_ext = nc.declare_dram_parameter("wk", [DM, OD], f32r, isOutput=False)
    wv_ext = nc.declare_dram_parameter("wv", [DM, OD], f32r, isOutput=False)
    wo_ext = nc.declare_dram_parameter("wo", [OD, DM], f32r, isOutput=False)
    bq_ext = nc.declare_dram_parameter("bq2", [128, OBLK], f32, isOutput=False)
    bk_ext = nc.declare_dram_parameter("bk2", [128, OBLK], f32, isOutput=False)
    bv_ext = nc.declare_dram_parameter("bvr", [1, OD], f32r, isOutput=False)
    bo_ext = nc.declare_dram_parameter("bo2", [1, DM], f32r, isOutput=False)
    mask_ext = nc.declare_dram_parameter("mask", [128, 128], f32, isOutput=False)
    ones_ext = nc.declare_dram_parameter("ones1", [1, 128], f32r, isOutput=False)
    out_ext = nc.declare_dram_parameter("out", [S // 2, DM], f32, isOutput=True)

    with tile.TileContext(nc) as tc, ExitStack() as ctx:
        consts = ctx.enter_context(tc.tile_pool(name="consts", bufs=1))
        persist = ctx.enter_context(tc.tile_pool(name="persist", bufs=1))
        xpool = ctx.enter_context(tc.tile_pool(name="xpool", bufs=12))
        wpool = ctx.enter_context(tc.tile_pool(name="wpool", bufs=12))
        wopool = ctx.enter_context(tc.tile_pool(name="wopool", bufs=1))
        ppool = ctx.enter_context(tc.tile_pool(name="ppool", bufs=4))
        stg = ctx.enter_context(tc.tile_pool(name="stg", bufs=2))
        ps_mm = ctx.enter_context(tc.tile_pool(name="ps_mm", bufs=2, space="PSUM"))
        ps_acc = ctx.enter_context(tc.tile_pool(name="ps_acc", bufs=2, space="PSUM"))
        ps_tr = ctx.enter_context(tc.tile_pool(name="ps_tr", bufs=2, space="PSUM"))
        dram = ctx.enter_context(tc.tile_pool(name="dram", bufs=1, space="DRAM"))

        # --- constants ---
        mask_sb = consts.tile([128, 128], f32, name="mask_sb")
        bq_sb = consts.tile([128, OBLK], f32, name="bq_sb")
        bk_sb = consts.tile([128, OBLK], f32, name="bk_sb")
        bv_sb = consts.tile([1, OD], f32r, name="bv_sb")
        bo_sb = consts.tile([1, DM], f32r, name="bo_sb")
        ones1 = consts.tile([1, 128], f32r, name="ones1")
        ones_col = consts.tile([128, NH, 1], f32, name="ones_col")
        nc.sync.dma_start(out=mask_sb, in_=mask_ext[:, :])
        nc.sync.dma_start(out=bq_sb, in_=bq_ext[:, :])
        nc.sync.dma_start(out=bk_sb, in_=bk_ext[:, :])
        nc.sync.dma_start(out=bv_sb, in_=bv_ext[:, :])
        nc.sync.dma_start(out=bo_sb, in_=bo_ext[:, :])
        nc.sync.dma_start(out=ones1, in_=ones_ext[:, :])
        nc.vector.memset(ones_col, 1.0)

        # --- persistent activations ---
        qT = [persist.tile([128, S], bf16, name=f"qT{i}") for i in range(OBLK)]
        kT = [persist.tile([128, S], bf16, name=f"kT{i}") for i in range(OBLK)]
        # v_sb[s]: [128, 8 heads * 65]; col 65h+64 is the ones column
        v_sb = [persist.tile([128, NH * (HD + 1)], bf16, name=f"v{s}") for s in range(SBLK)]
        ctxT = [persist.tile([128, S], f32r, name=f"ctxT{i}") for i in range(OBLK)]

        # --- output-projection weights (prefetch early) ---
        wo_sb = [wopool.tile([128, DM], f32r, name=f"wo{i}", tag=f"wo{i}") for i in range(OBLK)]
        for i in range(OBLK):
            nc.sync.dma_start(out=wo_sb[i], in_=wo_ext[i * 128:(i + 1) * 128, :])

        # ones columns of v
        for s in range(SBLK):
            v3 = v_sb[s].rearrange("p (h e) -> p h e", e=HD + 1)
            nc.vector.tensor_copy(out=v3[:, :, HD:HD + 1], in_=ones_col)

        # --- projection weights ---
        def load_w(w_ext, nm, x_ext=None, x_first=None):
            tiles = []
            for d in range(DBLK):
                t = wpool.tile([128, OD], f32r, name=f"{nm}{d}", tag="w")
                nc.sync.dma_start(out=t, in_=w_ext[d * 128:(d + 1) * 128, :])
                tiles.append(t)
                if x_ext is not None:
                    # interleave the first seq-chunk's x tile right behind its
                    # weight tile so matmul d can start as soon as both land
                    xt = xpool.tile([128, 512], f32r, name=f"{nm}xp{d}", tag="xt")
                    nc.sync.dma_start(
                        out=xt, in_=x_ext[d * 128:(d + 1) * 128, 0:512])
                    x_first.append(xt)
            return tiles

        cc_in = dram.tile([S, DM], f32, name="cc_in")
        cc_out = dram.tile([S // 2, DM], f32, name="cc_out")

        # --- q/k projections: qT[oblk][:, s] = (Wq.T @ x.T + bq) ---
        # psum tiles hold two 512-wide output blocks side by side (2 banks);
        # the bias-add copies run on DVE to keep ScalarE free for exp
        def proj_qk(x_ext, w_ext, bias_sb, dst, nm):
            # interleave the first x-chunk DMAs with the weight DMAs so the
            # first accumulation matmul starts after ~2 transfers, not 9
            x_first = []
            w_tiles = load_w(w_ext, nm + "w", x_ext, x_first)
            for sc in range(QCN):
                if sc == 0:
                    xt = x_first
                else:
                    xt = []
                    for d in range(DBLK):
                        t = xpool.tile([128, 512], f32r, name=f"{nm}x{sc}_{d}", tag="xt")
                        nc.sync.dma_start(
                            out=t, in_=x_ext[d * 128:(d + 1) * 128, sc * 512:(sc + 1) * 512])
                        xt.append(t)
                for obp in range(OBLK // 2):
                    psum = ps_mm.tile([128, 1024], f32, name=f"{nm}ps{sc}_{obp}", tag="mm")
                    for d in range(DBLK):
                        for half in range(2):
                            ob = 2 * obp + half
                            nc.tensor.matmul(
                                psum[:, 512 * half:512 * (half + 1)],
                                w_tiles[d][:, ob * 128:(ob + 1) * 128], xt[d],
                                start=(d == 0), stop=(d == DBLK - 1))
                    for half in range(2):
                        ob = 2 * obp + half
                        nc.vector.tensor_scalar_add(
                            out=dst[ob][:, sc * 512:(sc + 1) * 512],
                            in0=psum[:, 512 * half:512 * (half + 1)],
                            scalar1=bias_sb[:, ob:ob + 1])

        def _body():
            proj_qk(xq_ext, wq_ext, bq_sb, qT, "q")
            proj_qk(xk_ext, wk_ext, bk_sb, kT, "k")

            # --- v projection: v[s, o] = x @ Wv + bv (natural layout) ---
            wv_tiles = load_w(wv_ext, "vw")
            for sc in range(QCN):
                xt = []
                for d in range(DBLK):
                    t = xpool.tile([128, 512], f32r, name=f"vx{sc}_{d}", tag="xt")
                    nc.sync.dma_start(
                        out=t, in_=xv_ext[d * 128:(d + 1) * 128, sc * 512:(sc + 1) * 512])
                    xt.append(t)
                for sl in range(4):
                    s = sc * 4 + sl
                    psum = ps_mm.tile([128, 512], f32, name=f"vps{s}", tag="mm")
                    for d in range(DBLK):
                        nc.tensor.matmul(
                            psum, xt[d][:, sl * 128:(sl + 1) * 128], wv_tiles[d],
                            start=(d == 0), stop=False)
                    nc.tensor.matmul(psum, ones1, bv_sb, start=False, stop=True)
                    v3 = v_sb[s].rearrange("p (h e) -> p h e", e=HD + 1)
                    ps3 = psum.rearrange("p (h e) -> p h e", e=HD)
                    nc.vector.tensor_copy(out=v3[:, :, 0:HD], in_=ps3)

            # --- attention + output projection, chunk by chunk ---
            # Heads are processed in pairs (rows 0-63 / 64-127 of one kT/qT
            # tile): their K=64 scores matmuls sit in different PE row-groups
            # and run concurrently.
            def ctx_mm(acc, h, nkb, qc, pkb, pp):
                m = pkb - 4 * qc
                c0 = 128 * m if m > 0 else 0
                nc.tensor.matmul(
                    acc[:, c0:512], v_sb[pkb][:, 65 * h:65 * h + 65], pp[:, c0:512],
                    start=(pkb == 0), stop=(pkb == nkb - 1))

            def exp_pair(p, sps, m):
                # one strided activation covers both heads' live columns;
                # for diagonal blocks (m >= 0) only columns >= 128m are live
                c0 = 128 * m if m > 0 else 0
                if c0 == 0:
                    nc.scalar.activation(out=p, in_=sps, func=EXP)
                else:
                    p3 = p.rearrange("k (g q) -> k g q", q=512)
                    s3 = sps.rearrange("k (g q) -> k g q", q=512)
                    nc.scalar.activation(
                        out=p3[:, :, c0:512], in_=s3[:, :, c0:512], func=EXP)
                if m >= 0:
                    for j in range(2):
                        nc.vector.tensor_mul(
                            out=p[:, 512 * j + 128 * m:512 * j + 128 * (m + 1)],
                            in0=p[:, 512 * j + 128 * m:512 * j + 128 * (m + 1)],
                            in1=mask_sb)

            for qc in range(QCN):
                for hp in range(NH // 2):
                    t = hp
                    heads = (2 * hp, 2 * hp + 1)
                    nkb = 4 * qc + 4
                    lq = [qT[t][64 * j:64 * j + 64, qc * 512:(qc + 1) * 512]
                          for j in range(2)]
                    accs = [ps_acc.tile([HD + 1, 512], f32,
                                        name=f"acc{qc}_{hp}_{j}", tag="acc")
                            for j in range(2)]
                    pending = []
                    for kb in range(nkb):
                        m = kb - 4 * qc
                        c0 = 128 * m if m > 0 else 0
                        sps = ps_mm.tile([128, 1024], f32,
                                         name=f"s{qc}_{hp}_{kb}", tag="mm")
                        for j in range(2):
                            nc.tensor.matmul(
                                sps[:, 512 * j + c0:512 * (j + 1)],
                                kT[t][64 * j:64 * j + 64, kb * 128:(kb + 1) * 128],
                                lq[j][:, c0:512], start=True, stop=True)
                        p = ppool.tile([128, 1024], bf16,
                                       name=f"p{qc}_{hp}_{kb}", tag="p")
                        exp_pair(p, sps, m)
                        pending.append((kb, p))
                        # keep two block-pairs in flight so PE never waits on
                        # the newest blocks' exp
                        while len(pending) > 2:
                            pkb, pp = pending.pop(0)
                            for j in range(2):
                                ctx_mm(accs[j], heads[j], nkb, qc, pkb,
                                       pp[:, 512 * j:512 * (j + 1)])
                    while pending:
                        pkb, pp = pending.pop(0)
                        for j in range(2):
                            ctx_mm(accs[j], heads[j], nkb, qc, pkb,
                                   pp[:, 512 * j:512 * (j + 1)])

                    # normalize: ctxT[h] = acc[0:64] * broadcast(1/acc[64])
                    # (broadcast across partitions via a K=1 outer product)
                    for j in range(2):
                        h, r0 = heads[j], 64 * j
                        cstg = stg.tile([HD + 1, 512], f32,
                                        name=f"cstg{qc}_{hp}_{j}", tag="cstg")
                        nc.vector.tensor_copy(out=cstg, in_=accs[j])
                        rrow = stg.tile([1, 512], f32r, name=f"rr{qc}_{hp}_{j}",
                                        tag="rrow", bufs=4)
                        with nc.allow_low_precision(reason="f32r is fp32-width"):
                            nc.vector.reciprocal(out=rrow, in_=cstg[HD:HD + 1, :])
                        bc = ps_tr.tile([HD, 512], f32,
                                        name=f"bc{qc}_{hp}_{j}", tag="tr")
                        nc.tensor.matmul(bc, ones1[:, 0:HD], rrow,
                                         start=True, stop=True)
                        nc.vector.tensor_mul(
                            out=ctxT[t][r0:r0 + 64, qc * 512:(qc + 1) * 512],
                            in0=cstg[0:HD, :], in1=bc)

                # --- output projection + pairwise ReduceScatter for this
                # 512-row slab (overlaps the collective with later chunks) ---
                for s in range(4 * qc, 4 * qc + 4):
                    psum = ps_mm.tile([128, 1024], f32, name=f"ops{s}", tag="mm")
                    for nch in range(2):
                        sl = slice(512 * nch, 512 * (nch + 1))
                        for hb in range(OBLK):
                            nc.tensor.matmul(
                                psum[:, sl], ctxT[hb][:, s * 128:(s + 1) * 128],
                                wo_sb[hb][:, sl], start=(hb == 0), stop=False)
                        nc.tensor.matmul(
                            psum[:, sl], ones1, bo_sb[:, sl],
                            start=False, stop=True)
                    osb = stg.tile([128, 1024], f32, name=f"ob{s}",
                                   tag="osb", bufs=3)
                    # DVE, not ScalarE: keeps ACT free for the next chunk's exp
                    nc.vector.tensor_copy(out=osb, in_=psum)
                    nc.sync.dma_start(
                        out=cc_in[s * 128:(s + 1) * 128, :], in_=osb)
                nc.gpsimd.collective_compute(
                    "ReduceScatter", mybir.AluOpType.add,
                    replica_groups=[[0, 1], [2, 3], [4, 5], [6, 7]],
                    ins=[cc_in[qc * 512:(qc + 1) * 512, :].opt()],
                    outs=[cc_out[qc * 256:(qc + 1) * 256, :].opt()])
                nc.sync.dma_start(
                    out=out_ext[qc * 256:(qc + 1) * 256, :],
                    in_=cc_out[qc * 256:(qc + 1) * 256, :])

        # repeat>1 is a timing aid: one dispatch runs the body N times
        for _rep in range(repeat):
            _body()

    nc.finalize()
    return nc


def _get_program(repeat=1):
    global _prog
    if repeat != 1:
        return _build_program(repeat)
    if _prog is None:
        _prog = _build_program()
    return _prog


def make_in_maps(query, key, value, Wq, bq, Wk, bk, Wv, bv, Wo, bo):
    query = np.asarray(query, dtype=np.float32)
    key = np.asarray(key, dtype=np.float32)
    value = np.asarray(value, dtype=np.float32)
    Wq = np.asarray(Wq, dtype=np.float32)
    bq = np.asarray(bq, dtype=np.float32)
    Wk = np.asarray(Wk, dtype=np.float32)
    bk = np.asarray(bk, dtype=np.float32)
    Wv = np.asarray(Wv, dtype=np.float32)
    bv = np.asarray(bv, dtype=np.float32)
    Wo = np.asarray(Wo, dtype=np.float32)
    bo = np.asarray(bo, dtype=np.float32)

    scale = 1.0 / np.sqrt(np.float32(DM))
    mask = np.triu(np.ones((128, 128), dtype=np.float32))

    xq_t = [np.ascontiguousarray(query[b].T) for b in range(B)]
    xk_t = [np.ascontiguousarray(key[b].T) for b in range(B)]
    xv_t = [np.ascontiguousarray(value[b].T) for b in range(B)]

    in_maps = []
    for c in range(NCORES):
        b, hh = c // 2, c % 2
        cols = slice(hh * OD, (hh + 1) * OD)
        in_maps.append({
            "xq": xq_t[b],
            "xk": xk_t[b],
            "xv": xv_t[b],
            "wq": np.ascontiguousarray(Wq[:, cols] * scale),
            "wk": np.ascontiguousarray(Wk[:, cols]),
            "wv": np.ascontiguousarray(Wv[:, cols]),
            "wo": np.ascontiguousarray(Wo[cols, :]),
            "bq2": np.ascontiguousarray((bq[cols] * scale).reshape(OBLK, 128).T),
            "bk2": np.ascontiguousarray(bk[cols].reshape(OBLK, 128).T),
            "bvr": np.ascontiguousarray(bv[cols].reshape(1, OD)),
            "bo2": np.ascontiguousarray((bo / 2.0).reshape(1, DM)),
            "mask": mask,
            "ones1": np.ones((1, 128), dtype=np.float32),
        })
    return in_maps


def assemble(core_outs):
    """core_outs[c]: [S//2, DM]; slab qc rows [256qc,256qc+256) are global rows
    [512qc, 512qc+256) for even cores, [512qc+256, 512qc+512) for odd."""
    out = np.empty((B, S, DM), dtype=np.float32)
    for b in range(B):
        ev, od = core_outs[2 * b], core_outs[2 * b + 1]
        for qc in range(QCN):
            out[b, 512 * qc:512 * qc + 256] = ev[256 * qc:256 * (qc + 1)]
            out[b, 512 * qc + 256:512 * (qc + 1)] = od[256 * qc:256 * (qc + 1)]
    return out


def kernel(query, key, value, Wq, bq, Wk, bk, Wv, bv, Wo, bo):
    import time
    from concourse.bass_utils import run_bass_kernel_spmd

    in_maps = make_in_maps(query, key, value, Wq, bq, Wk, bk, Wv, bv, Wo, bo)
    nc = _get_program()
    try:
        res = run_bass_kernel_spmd(nc, in_maps, list(range(NCORES)))
    except Exception:
        time.sleep(10)  # transient device errors recover on retry
        res = run_bass_kernel_spmd(nc, in_maps, list(range(NCORES)))
    return assemble([res.results[c]["out"] for c in range(NCORES)])

